# revision 18
# baseline (speedup 1.0000x reference)
"""Single-head causal attention (B=8, T=2048, C=1024, head_dim=64) on 8 TRN2 NeuronCores.

Sharding: data-parallel over batch -- one batch element per core, qkv weights
replicated. Host prep per core: x[b] transposed to [C, T] fp16 and packed into
16 DMA-contiguous half-slabs xt[h*8+j] = x^T[128j:128j+128, 1024h:1024h+1024];
W pre-packed into SBUF chunk layout; mask/identity constants from host.

v5 schedule: group-pipelined. T is split into four 512-col groups; the PE
stream is warm, proj(0), proj(1), attn(0), proj(2), attn(1), proj(3),
attn(2), attn(3), so attention fills projection's DMA-paced gaps, the qT
shift DMA of group g completes under proj(g+1), and the PE stays busy enough
to hold its ramped clock (TRN2 tensor clock: 0.65/1.2/2.4 GHz by sustained
use). v [h, s] -> [s, h] transposes go through the DMA XBAR (16 dma
transposes) instead of the PE. Attention is software-pipelined in PAIRS of
128-row s-chunks: two STs share a [128, 1024] PSUM tile and (off-diagonal)
one exp activation call; the PV pair of pair p-1 runs while exp(p) cooks.

Output leaves the device UNNORMALIZED as out[g] = [65, 512] fp16 per group
(rows 0:64 = out^T, row 64 = softmax denominator). Final divide + transpose
happens on host; removes all output-side PE transposes/reciprocals and makes
each group's store one contiguous DMA.
"""

import numpy as np

import concourse.bass as bass
import concourse.mybir as mybir
from concourse import bacc
from concourse.bass import ts
from concourse.bass_utils import run_bass_kernel_spmd
from concourse.tile import TileContext

B, T, C = 8, 2048, 1024
HD = 64
N_CORES = 8
NJ = C // 128  # contraction chunks for the qkv projection
NT = T // 128  # 128-row tiles along T
NG = T // 512  # 512-col groups along T
FP16 = mybir.dt.float16
F32 = mybir.dt.float32
EXP = mybir.ActivationFunctionType.Exp
CST_W = NJ * 192 + 2 + 128 + 128  # 1794


def build_nc() -> bass.Bass:
    nc = bacc.Bacc(None, target_bir_lowering=False)
    # half-slabs, each contiguous: xt[h*8+j] = x^T[128j:128(j+1), 1024h:1024(h+1)]
    xt = nc.declare_dram_parameter("xt", [16, 128, 1024], FP16, isOutput=False)
    # cst per partition: NJ*192 w-chunk cols | bkq | bv | msk | idh
    cst = nc.declare_dram_parameter("cst", [128, CST_W], FP16, isOutput=False)
    # unnormalized transposed output per group: rows 0:64 = out^T, row 64 = denom
    out = nc.declare_dram_parameter("out", [NG, 65, 512], FP16, isOutput=True)

    with TileContext(nc) as tc:
        with (
            tc.tile_pool(name="consts", bufs=1) as consts,
            tc.tile_pool(name="xtp", bufs=NJ) as xtp,
            tc.tile_pool(name="kqv", bufs=1) as kqv,
            tc.tile_pool(name="vtp", bufs=2) as vtp,
            tc.tile_pool(name="ptp", bufs=3) as ptp,
            tc.tile_pool(name="epi", bufs=2) as epi,
            tc.tile_pool(name="ppk", bufs=2, space=bass.MemorySpace.PSUM) as ppk,
            tc.tile_pool(name="ppv", bufs=1, space=bass.MemorySpace.PSUM) as ppv,
            tc.tile_pool(name="pst", bufs=2, space=bass.MemorySpace.PSUM) as pst,
            tc.tile_pool(name="pso", bufs=1, space=bass.MemorySpace.PSUM) as pso,
        ):
            # --- constants first: w gates every projection matmul ---
            cst_sb = consts.tile([128, CST_W], FP16)
            nc.sync.dma_start(out=cst_sb[:], in_=cst[:, :])
            w_sb = cst_sb  # cols j*192 + [0:128) = Wkq_j, + [128:192) = Wv_j
            msk_sb = cst_sb[:, 1538:1666]
            idh_sb = cst_sb[:, 1666:1794]

            # --- x^T half-slabs; j0h0 leads the scalar queue so the first kq
            # matmul isn't stuck behind cst on sync. Issues spread over three
            # sequencers so descriptor generation parallelizes. ---
            xts = []
            for j in range(NJ):
                xts.append(xtp.tile([128, T], FP16, tag="xt", name=f"xt{j}"))
            dma_plan = [
                (nc.scalar, [0, 3, 6, 8, 11, 14]),
                (nc.sync, [1, 4, 7, 10, 13]),
                (nc.gpsimd, [2, 5, 9, 12, 15]),
            ]
            for eng, slabs in dma_plan:
                for s in slabs:
                    h, j = divmod(s, 8)
                    eng.dma_start(out=xts[j][:, ts(h, 1024)], in_=xt[s, :, :])

            # --- SBUF state ---
            wu_sb = consts.tile([1, 256], FP16)
            nc.vector.memset(wu_sb[:], 1.0)
            bias32 = consts.tile([128, 2], F32)
            nc.vector.tensor_copy(bias32[:, 0:1], cst_sb[:, 1536:1537])
            nc.vector.tensor_copy(bias32[0:64, 1:2], cst_sb[0:64, 1537:1538])
            kqT = kqv.tile([128, T], FP16)
            qT = kqv.tile([64, T], FP16)
            v1 = kqv.tile([128, NT, 80], FP16)  # [s, hd | ones | pad] per t-tile
            # vT buffers: rows 0:64 = v^T (per group), row 64 = ones (so the
            # PE transpose directly yields [s, v|1]), rows 65:128 zero.
            vts = [vtp.tile([128, 512], FP16, tag="vt", name=f"vt{i}") for i in range(2)]
            for vt in vts:
                nc.vector.memset(vt[64:128, :], 0.0)
                nc.vector.memset(vt[64:65, :], 1.0)

            wu_ps = pst.tile([128, 256], F32, tag="st", name="wu_ps")

            def warm(n):
                for _ in range(n):
                    nc.tensor.matmul(
                        wu_ps[:], wu_sb[:, 0:128], wu_sb[:], start=True, stop=True
                    )

            def proj(n, warm_per_j=0):
                kq_acc = ppk.tile([128, 512], F32, tag="kq", name=f"kq_acc{n}")
                v_acc = ppv.tile([64, 512], F32, tag="v", name=f"v_acc{n}")
                for j in range(NJ):
                    nc.tensor.matmul(
                        kq_acc[:], w_sb[:, j * 192:j * 192 + 128],
                        xts[j][:, ts(n, 512)], start=(j == 0), stop=(j == NJ - 1),
                    )
                    if warm_per_j:
                        warm(warm_per_j)
                nc.vector.tensor_scalar_add(kqT[:, ts(n, 512)], kq_acc[:], bias32[:, 0:1])
                nc.gpsimd.dma_start(out=qT[:, ts(n, 512)], in_=kqT[64:128, ts(n, 512)])
                for j in range(NJ):
                    nc.tensor.matmul(
                        v_acc[:], w_sb[:, j * 192 + 128:j * 192 + 192],
                        xts[j][:, ts(n, 512)], start=(j == 0), stop=(j == NJ - 1),
                    )
                    if warm_per_j:
                        warm(warm_per_j)
                nc.vector.tensor_scalar_add(
                    vts[n % 2][0:64, :], v_acc[:], bias32[0:64, 1:2]
                )

            def tpose(n):
                tpv = pso.tile([128, 4, 128], FP16, tag="o", name=f"tpv{n}")
                for k in range(4):
                    i = 4 * n + k
                    nc.tensor.transpose(tpv[:, k, :], vts[n % 2][:, ts(k, 128)], idh_sb)
                    nc.vector.tensor_copy(v1[:, i, 0:HD + 1], tpv[:, k, 0:HD + 1])

            def attn(g):
                gb = 512 * g
                jmax = 4 * g + 3
                acc = pso.tile([65, 512], F32, tag="o", name=f"acc{g}")
                eo = epi.tile([65, 512], FP16, tag="eo", name=f"eo{g}")
                pend = []  # software pipeline: PV pair trails ST/exp pair by one

                def do_pv(jj, a, ptt, col):
                    lo = col + a - gb
                    if jj >= 4 * g:  # diagonal block: keep s <= t
                        nc.vector.tensor_mul(ptt[:, lo:lo + 128], ptt[:, lo:lo + 128], msk_sb)
                    nc.tensor.matmul(
                        acc[:, a - gb:512], v1[:, jj, 0:65], ptt[:, lo:col + 512],
                        start=(jj == 0), stop=(jj == jmax),
                    )

                def flush_pair():
                    q, a1, a2, pt = pend.pop(0)
                    do_pv(2 * q, a1, pt, 0)
                    do_pv(2 * q + 1, a2, pt, 512)
                    if g == NG - 1 and q == 2 * g:
                        # cols [0:256) are final after PV(jmax-2): stream the
                        # front of the last group out while the tail finishes
                        nc.vector.tensor_copy(eo[:, 0:256], acc[:, 0:256])
                        nc.sync.dma_start(out=out[g, :, 0:256], in_=eo[:, 0:256])

                for p in range(2 * g + 2):
                    jA, jB = 2 * p, 2 * p + 1
                    aA, aB = max(128 * jA, gb), max(128 * jB, gb)
                    stp = pst.tile([128, 1024], F32, tag="st", name=f"stp{g}_{p}")
                    ptt = ptp.tile([128, 1024], FP16, tag="pt", name=f"ptt{g}_{p}")
                    for jj, a, col in ((jA, aA, 0), (jB, aB, 512)):
                        nc.tensor.matmul(
                            stp[:, col + a - gb:col + 512],
                            kqT[0:64, ts(jj, 128)], qT[:, a:gb + 512],
                            start=True, stop=True,
                        )
                    if jB >= 4 * g:  # diagonal pair: separate exp per chunk
                        for jj, a, col in ((jA, aA, 0), (jB, aB, 512)):
                            nc.scalar.activation(
                                ptt[:, col + a - gb:col + 512],
                                stp[:, col + a - gb:col + 512], EXP, scale=0.125,
                            )
                    else:
                        nc.scalar.activation(ptt[:], stp[:], EXP, scale=0.125)
                    pend.append((p, aA, aB, ptt))
                    if len(pend) > 1:
                        flush_pair()
                while pend:
                    flush_pair()
                if g == NG - 1:
                    nc.vector.tensor_copy(eo[:, 256:512], acc[:, 256:512])
                    nc.sync.dma_start(out=out[g, :, 256:512], in_=eo[:, 256:512])
                else:
                    nc.vector.tensor_copy(eo[:], acc[:])
                    eng = nc.sync if g % 2 == 0 else nc.gpsimd
                    eng.dma_start(out=out[g, :, :], in_=eo[:])

            # --- pipelined schedule: attn(g) is emitted after proj(g+1) so
            # the qT(g) shift DMA completes while the PE runs proj(g+1) ---
            warm(12)
            proj(0, warm_per_j=2)
            proj(1)
            tpose(0)
            attn(0)
            proj(2)
            tpose(1)
            attn(1)
            proj(3)
            tpose(2)
            attn(2)
            tpose(3)
            attn(3)
    nc.compile()
    return nc


_NC_CACHE = None


def _get_nc() -> bass.Bass:
    global _NC_CACHE
    if _NC_CACHE is None:
        _NC_CACHE = build_nc()
    return _NC_CACHE


def make_in_maps(x: np.ndarray, W: np.ndarray, b: np.ndarray) -> list[dict]:
    cst = np.zeros((128, CST_W), dtype=np.float16)
    # w chunks: cst[p, j*192+m] = W[j*128+p, m]
    cst[:, :NJ * 3 * HD] = (
        W.astype(np.float16).reshape(NJ, 128, 3 * HD).transpose(1, 0, 2).reshape(128, NJ * 3 * HD)
    )
    cst[:, 1536] = b[0:128].astype(np.float16)
    cst[0:64, 1537] = b[128:192].astype(np.float16)
    cst[:, 1538:1666] = np.triu(np.ones((128, 128), dtype=np.float16))  # keep s <= t
    cst[:, 1666:1794] = np.eye(128, dtype=np.float16)
    cst = np.ascontiguousarray(cst)
    in_maps = []
    for core in range(N_CORES):
        xtc = x[core].astype(np.float16).T  # [C, T]
        # [16, 128, 1024]: slab h*8+j = xtc[128j:128(j+1), 1024h:1024(h+1)]
        slabs = (
            xtc.reshape(NJ, 128, 2, 1024).transpose(2, 0, 1, 3).reshape(16, 128, 1024)
        )
        in_maps.append({"xt": np.ascontiguousarray(slabs), "cst": cst})
    return in_maps


def _unshard(raw: np.ndarray) -> np.ndarray:
    # raw [NG, 65, 512] per group; rows 0:64 = out^T unnormalized, row 64 = denom
    o = np.concatenate([raw[g] for g in range(NG)], axis=1).astype(np.float32)
    return (o[0:HD, :] / o[HD:HD + 1, :]).T


def run(x, W, b, trace: bool = False):
    """Returns (output [B, T, HD] fp32, BassKernelResults)."""
    x, W, b = np.asarray(x), np.asarray(W), np.asarray(b)
    nc = _get_nc()
    res = run_bass_kernel_spmd(nc, make_in_maps(x, W, b), list(range(N_CORES)), trace=trace)
    out = np.stack([_unshard(res.results[i]["out"]) for i in range(N_CORES)], axis=0)
    return out.astype(np.float32), res


def kernel(x, W, b) -> np.ndarray:
    out, _ = run(x, W, b)
    return out
